# revision 18
# baseline (speedup 1.0000x reference)
"""GCNConv Trainium2 kernel.

out = relu(deg^-1/2 ⊙ (A_w @ (deg^-1/2 ⊙ (x @ W)))) with self-loops, per graph.

Sharding: 8 cores = 4 graphs x 2 destination-row halves. Inside each core the
graph op is an SpMM executed as: dma_gather of y rows per edge (edges land on
partitions), a DVE-built w-valued one-hot scatter matrix per 128-edge chunk,
and a PE matmul accumulating each 128-row destination block in PSUM.
"""

import sys

sys.path.insert(0, "/opt/trn_rl_repo")

import numpy as np
import ml_dtypes

B, N, C, E = 4, 10000, 128, 160000
P = 128
NBLK = 80            # padded row blocks (10240 rows) for degree tables
NPAD = NBLK * P      # 10240
NBLK_Y = 79          # y table blocks (10112 rows >= N)
NPAD_Y = NBLK_Y * P  # 10112
HALF = 40            # destination blocks per core
HROWS = HALF * P     # 5120
GBLK = 1             # blocks per dma_gather batch

_COMPILED = {}       # (L, CB) -> (nc, in_names)


def _build(L, CB, repeat=1, rep_main_only=False, mode="full", nqueues=4,
           gblk=GBLK, single_packet=False, dt16=True, ydt32=False):
    import concourse.bacc as bacc
    import concourse.mybir as mybir
    from concourse import tile

    dt = mybir.dt
    fdt = dt.bfloat16 if dt16 else dt.float32
    ydt = dt.float32 if ydt32 else fdt
    TC = HALF * CB * P          # padded edge slots per core
    NCHUNK = HALF * CB
    G_CH = gblk * CB            # chunks per gather batch
    NBATCH = HALF // gblk
    do_gather = mode in ("full", "gather_only")
    do_compute = mode in ("full", "compute_only")

    nc = bacc.Bacc(
        "TRN2", target_bir_lowering=True, debug=False, num_swdge_queues=nqueues
    )
    xT = nc.dram_tensor("xT", [P, NPAD_Y], fdt, kind="ExternalInput")
    wmat = nc.dram_tensor("wmat", [P, P], fdt, kind="ExternalInput")
    wpc = nc.dram_tensor("wpc", [P, NBLK * L], dt.float32, kind="ExternalInput")
    wpl = nc.dram_tensor("wpl", [P, HALF * L], dt.float32, kind="ExternalInput")
    idx16 = nc.dram_tensor("idx16", [P, TC // 16], dt.int16, kind="ExternalInput")
    rowloc = nc.dram_tensor("rowloc", [P, NCHUNK], dt.float32, kind="ExternalInput")
    wve = nc.dram_tensor("wve", [P, NCHUNK], dt.float32, kind="ExternalInput")
    iota = nc.dram_tensor("iota", [P, P], fdt, kind="ExternalInput")
    outd = nc.dram_tensor("outd", [HROWS, P], dt.float32, kind="ExternalOutput")

    with tile.TileContext(nc) as tc:
        with (
            tc.tile_pool(name="const", bufs=1) as cp,
            tc.tile_pool(name="ystage", bufs=2) as yp,
            tc.tile_pool(name="xstage", bufs=2) as xp,
            tc.tile_pool(name="ostage", bufs=2) as op,
            tc.tile_pool(name="gather", bufs=max(2, 10 // gblk)) as gp,
            tc.tile_pool(name="onehot", bufs=4) as ohp,
            tc.tile_pool(name="psxw", bufs=4, space="PSUM") as pxw,
            tc.tile_pool(name="psmain", bufs=2, space="PSUM") as pmain,
            tc.tile_pool(name="dram", bufs=1, space="DRAM") as dp,
        ):
            wmat_sb = cp.tile([P, P], fdt)
            wpc_sb = cp.tile([P, NBLK * L], dt.float32)
            wpl_sb = cp.tile([P, HALF * L], dt.float32)
            idx_sb = cp.tile([P, TC // 16], dt.int16)
            rl_sb = cp.tile([P, NCHUNK], dt.float32)
            wve_sb = cp.tile([P, NCHUNK], dt.float32)
            iota_sb = cp.tile([P, P], fdt)
            deg_c = cp.tile([P, NBLK], dt.float32)
            dinv_c = cp.tile([P, NBLK], dt.float32)
            deg_l = cp.tile([P, HALF], dt.float32)
            dinv_l = cp.tile([P, HALF], dt.float32)

            for rep in range(1 if rep_main_only else repeat):
                nc.sync.dma_start(out=wmat_sb[:], in_=wmat[:])
                nc.sync.dma_start(out=wpc_sb[:], in_=wpc[:])
                nc.sync.dma_start(out=wpl_sb[:], in_=wpl[:])
                nc.sync.dma_start(out=idx_sb[:], in_=idx16[:])
                nc.sync.dma_start(out=rl_sb[:], in_=rowloc[:])
                nc.sync.dma_start(out=wve_sb[:], in_=wve[:])
                nc.sync.dma_start(out=iota_sb[:], in_=iota[:])

                # weighted degree (self-loops included in the tables) and rsqrt
                nc.vector.tensor_reduce(
                    deg_c[:],
                    wpc_sb[:].rearrange("p (b l) -> p b l", l=L),
                    mybir.AxisListType.X,
                    mybir.AluOpType.add,
                )
                nc.vector.tensor_reduce(
                    deg_l[:],
                    wpl_sb[:].rearrange("p (b l) -> p b l", l=L),
                    mybir.AxisListType.X,
                    mybir.AluOpType.add,
                )
                sq_c = cp.tile([P, NBLK], dt.float32)
                sq_l = cp.tile([P, HALF], dt.float32)
                nc.scalar.activation(
                    sq_c[:], deg_c[:], mybir.ActivationFunctionType.Sqrt
                )
                nc.scalar.activation(
                    sq_l[:], deg_l[:], mybir.ActivationFunctionType.Sqrt
                )
                nc.vector.reciprocal(dinv_c[:], sq_c[:])
                nc.vector.reciprocal(dinv_l[:], sq_l[:])

                y_dram = dp.tile([NPAD_Y, P], ydt)

                # y = deg^-1/2 * (x @ W), written to DRAM in groups of 8 blocks
                for g0 in range(0, NBLK_Y, 8):
                    nb = min(8, NBLK_Y - g0)
                    ystage = yp.tile([P, 8 * P], ydt, tag="ystage")
                    xs = xp.tile([P, 8 * P], fdt, tag="xstage")
                    nc.sync.dma_start(
                        out=xs[:, : nb * P], in_=xT[:, g0 * P : (g0 + nb) * P]
                    )
                    for j in range(nb):
                        a = g0 + j
                        ps = pxw.tile([P, P], dt.float32)
                        nc.tensor.matmul(
                            ps[:],
                            lhsT=xs[:, j * P : (j + 1) * P],
                            rhs=wmat_sb[:],
                            start=True,
                            stop=True,
                        )
                        nc.scalar.activation(
                            ystage[:, j * P : (j + 1) * P],
                            ps[:],
                            mybir.ActivationFunctionType.Copy,
                            scale=dinv_c[:, a : a + 1],
                        )
                    nc.sync.dma_start(
                        out=y_dram[g0 * P : (g0 + nb) * P, :].rearrange(
                            "(j p) c -> p j c", p=P
                        ),
                        in_=ystage[:, : nb * P].rearrange("p (j c) -> p j c", c=P),
                    )

                # main loop: gather 2 blocks worth of edges at a time, scatter via
                # one-hot matmul into a per-block PSUM accumulator
                ostage = None
                gbuf = None
                if not do_gather:
                    gbuf = cp.tile([P, G_CH, P], ydt)
                    for j in range(G_CH):
                        nc.sync.dma_start(
                            out=gbuf[:, j, :],
                            in_=y_dram[j * P : (j + 1) * P, :],
                        )
                for _mrep in range(repeat if rep_main_only else 1):
                  for b in range(HALF):
                      if b % gblk == 0 and do_gather:
                          gbuf = gp.tile([P, G_CH, P], ydt, tag="gbuf")
                          bb = b // gblk
                          nc.gpsimd.dma_gather(
                              gbuf[:],
                              y_dram[:],
                              idx_sb[:, bb * G_CH * 8 : (bb + 1) * G_CH * 8],
                              G_CH * P,
                              G_CH * P,
                              P,
                              single_packet=single_packet,
                              queue_num=bb % nqueues,
                          )
                      if not do_compute:
                          continue
                      ps = pmain.tile([P, P], dt.float32, tag="psmain")
                      for k in range(CB):
                          ch = b * CB + k
                          slot = (b % gblk) * CB + k
                          oh = ohp.tile([P, P], fdt, tag="onehot")
                          nc.vector.tensor_scalar(
                              oh[:],
                              iota_sb[:],
                              rl_sb[:, ch : ch + 1],
                              wve_sb[:, ch : ch + 1],
                              mybir.AluOpType.is_equal,
                              mybir.AluOpType.mult,
                          )
                          nc.tensor.matmul(
                              ps[:],
                              lhsT=oh[:],
                              rhs=gbuf[:, slot, :],
                              start=(k == 0),
                              stop=(k == CB - 1),
                          )
                      if b % 8 == 0:
                          ostage = op.tile([P, 8 * P], dt.float32, tag="ostage")
                      nc.scalar.activation(
                          ostage[:, (b % 8) * P : (b % 8 + 1) * P],
                          ps[:],
                          mybir.ActivationFunctionType.Relu,
                          scale=dinv_l[:, b : b + 1],
                      )
                      if b % 8 == 7:
                          g0 = b - 7
                          nc.sync.dma_start(
                              out=outd[g0 * P : (g0 + 8) * P, :].rearrange(
                                  "(j p) c -> p j c", p=P
                              ),
                              in_=ostage[:].rearrange("p (j c) -> p j c", c=P),
                          )
    nc.compile()
    return nc


def _get(L, CB, repeat=1, rep_main_only=False, **kw):
    key = (L, CB, repeat, rep_main_only, tuple(sorted(kw.items())))
    if key not in _COMPILED:
        _COMPILED[key] = _build(L, CB, repeat, rep_main_only, **kw)
    return _COMPILED[key]


def _prep_inputs(x, edge_index, edge_weight, weight, dt16=True):
    """Host prep: per-graph balanced destination-row permutation + tables.

    Rows are assigned to the 80 destination blocks by in-degree serpentine
    (sorted desc, snake across blocks) so per-block edge counts are nearly
    equal -> CB (chunks per block) drops to ~ceil(mean/128). Sources within
    each block are sorted ascending for HBM gather locality.
    """
    bf16 = ml_dtypes.bfloat16
    fdt = bf16 if dt16 else np.float32
    x = np.asarray(x, np.float32)
    ei = np.asarray(edge_index)
    ew = np.asarray(edge_weight, np.float32)
    wt = np.asarray(weight, np.float32)

    loops = np.arange(N, dtype=np.int64)
    ones = np.ones(N, np.float32)
    NB = 2 * HALF

    graphs = []
    L_glob = 1
    CB_glob = 1
    for g in range(B):
        rows = np.concatenate([ei[g, 0], loops]).astype(np.int64)
        cols = np.concatenate([ei[g, 1], loops]).astype(np.int64)
        w = np.concatenate([ew[g], ones])

        counts = np.bincount(rows, minlength=N).astype(np.int64)
        L = int(counts.max())
        L_glob = max(L_glob, L)

        # serpentine balanced assignment of rows -> (block, slot)
        order = np.argsort(-counts, kind="stable")
        i = np.arange(N)
        rnd, pos = i // NB, i % NB
        blk_srt = np.where(rnd % 2 == 0, pos, NB - 1 - pos)
        blk_of_row = np.empty(N, np.int64)
        slot_of_row = np.empty(N, np.int64)
        blk_of_row[order] = blk_srt
        slot_of_row[order] = rnd

        eb = blk_of_row[rows]
        rl_full = slot_of_row[rows]

        halves = []
        for h in range(2):
            m = (eb >= h * HALF) & (eb < (h + 1) * HALF)
            lb = eb[m] - h * HALF
            hc = cols[m]
            hw = w[m]
            hrl = rl_full[m]
            o2 = np.lexsort((hc, lb))
            lb, hc, hw, hrl = lb[o2], hc[o2], hw[o2], hrl[o2]
            cnt = np.bincount(lb, minlength=HALF)
            CB_glob = max(CB_glob, int(np.ceil(cnt.max() / P)))
            halves.append((lb, hc, hw, hrl, cnt))
        graphs.append((rows, w, counts, halves, blk_of_row, slot_of_row))

    L, CB = L_glob, CB_glob
    TC = HALF * CB * P

    iota_np = np.tile(np.arange(P, dtype=np.float32), (P, 1)).astype(fdt)
    perms = []

    in_maps = []
    for g in range(B):
        rows, w, counts, halves, blk_of_row, slot_of_row = graphs[g]
        perms.append(blk_of_row * P + slot_of_row)
        # canonical degree table [NPAD, L] (source-row order, for dinv_c)
        order = np.argsort(rows, kind="stable")
        starts = np.zeros(N + 1, np.int64)
        np.cumsum(counts, out=starts[1:])
        srt_rows = rows[order]
        slot = np.arange(rows.size, dtype=np.int64) - starts[srt_rows]
        wpad = np.zeros((NPAD, L), np.float32)
        wpad[srt_rows, slot] = w[order]
        wpad[N:, 0] = 1.0
        wpc = (
            wpad.reshape(NBLK, P, L).transpose(1, 0, 2).reshape(P, NBLK * L).copy()
        )

        xp = np.zeros((NPAD_Y, P), np.float32)
        xp[:N] = x[g]
        xT = np.ascontiguousarray(xp.T).astype(fdt)

        for h in range(2):
            lb, hc, hw, hrl, cnt = halves[h]
            # permuted destination degree table for dinv_l
            ris = np.full(HROWS, -1, np.int64)
            sel = (blk_of_row >= h * HALF) & (blk_of_row < (h + 1) * HALF)
            rr = np.nonzero(sel)[0]
            ris[(blk_of_row[rr] - h * HALF) * P + slot_of_row[rr]] = rr
            wpl_rows = np.zeros((HROWS, L), np.float32)
            valid = ris >= 0
            wpl_rows[valid] = wpad[ris[valid]]
            wpl_rows[~valid, 0] = 1.0
            wpl = (
                wpl_rows.reshape(HALF, P, L)
                .transpose(1, 0, 2)
                .reshape(P, HALF * L)
                .copy()
            )
            bstarts = np.zeros(HALF + 1, np.int64)
            np.cumsum(cnt, out=bstarts[1:])
            dst = lb * (CB * P) + (np.arange(lb.size, dtype=np.int64) - bstarts[lb])
            cols_pad = np.zeros(TC, np.int64)
            wv_pad = np.zeros(TC, np.float32)
            rl_pad = np.zeros(TC, np.float32)
            cols_pad[dst] = hc
            wv_pad[dst] = hw
            rl_pad[dst] = hrl.astype(np.float32)

            idx16 = np.tile(
                cols_pad.astype(np.int16).reshape(-1, 16).T, (8, 1)
            ).copy()
            rl_sb = np.ascontiguousarray(rl_pad.reshape(-1, P).T)
            wve_sb = np.ascontiguousarray(wv_pad.reshape(-1, P).T)

            in_maps.append(
                {
                    "xT": xT,
                    "wmat": wt.astype(fdt),
                    "wpc": wpc,
                    "wpl": wpl,
                    "idx16": idx16,
                    "rowloc": rl_sb,
                    "wve": wve_sb,
                    "iota": iota_np,
                }
            )
    return in_maps, L, CB, perms


_RUNNERS = {}


def _make_runner(nc):
    """Persistent jitted 8-core SPMD runner for a compiled Bass module.

    Mirrors bass2jax.run_bass_via_pjrt's multi-core path, but reusable
    across calls so repeated executions don't re-trace.
    """
    import jax
    import jax.numpy as jnp
    import concourse.mybir as mybir
    from jax.sharding import Mesh, PartitionSpec
    from jax.experimental.shard_map import shard_map
    from concourse.bass2jax import (
        _bass_exec_p,
        install_neuronx_cc_hook,
        partition_id_tensor,
    )

    install_neuronx_cc_hook()
    n_cores = 8
    pname = nc.partition_id_tensor.name if nc.partition_id_tensor else None
    in_names, out_names, out_avals = [], [], []
    for alloc in nc.m.functions[0].allocations:
        if not isinstance(alloc, mybir.MemoryLocationSet):
            continue
        name = alloc.memorylocations[0].name
        if alloc.kind == "ExternalInput":
            if name != pname:
                in_names.append(name)
        elif alloc.kind == "ExternalOutput":
            out_names.append(name)
            out_avals.append(
                jax.core.ShapedArray(
                    tuple(alloc.tensor_shape), mybir.dt.np(alloc.dtype)
                )
            )
    n_params = len(in_names)
    all_names = in_names + out_names
    if pname is not None:
        all_names = all_names + [pname]

    def _body(*args):
        operands = list(args)
        if pname is not None:
            operands.append(partition_id_tensor())
        return tuple(
            _bass_exec_p.bind(
                *operands,
                out_avals=tuple(out_avals),
                in_names=tuple(all_names),
                out_names=tuple(out_names),
                lowering_input_output_aliases=(),
                sim_require_finite=True,
                sim_require_nnan=True,
                nc=nc,
            )
        )

    devices = jax.devices()[:n_cores]
    mesh = Mesh(np.asarray(devices), ("core",))
    nz = len(out_avals)
    donate = tuple(range(n_params, n_params + nz))
    sharded = jax.jit(
        shard_map(
            _body,
            mesh=mesh,
            in_specs=(PartitionSpec("core"),) * (n_params + nz),
            out_specs=(PartitionSpec("core"),) * nz,
            check_rep=False,
        ),
        donate_argnums=donate,
        keep_unused=True,
    )

    def run(in_maps, want_np=True):
        concat_in = [
            np.concatenate([np.asarray(m[name]) for m in in_maps], axis=0)
            for name in in_names
        ]
        zeros = [
            jnp.zeros((n_cores * a.shape[0], *a.shape[1:]), a.dtype)
            for a in out_avals
        ]
        outs = sharded(*concat_in, *zeros)
        if not want_np:
            return outs
        return [
            {
                name: np.asarray(outs[i]).reshape(n_cores, *out_avals[i].shape)[c]
                for i, name in enumerate(out_names)
            }
            for c in range(n_cores)
        ]

    run.in_names = in_names
    run.out_avals = out_avals
    run.sharded = sharded
    run.n_params = n_params
    return run


def _get_runner(L, CB, repeat=1, **kw):
    key = (L, CB, repeat, tuple(sorted(kw.items())))
    if key not in _RUNNERS:
        _RUNNERS[key] = _make_runner(_get(L, CB, repeat, **kw))
    return _RUNNERS[key]


def kernel(x, edge_index, edge_weight, weight):
    in_maps, L, CB, perms = _prep_inputs(x, edge_index, edge_weight, weight)
    run = _get_runner(L, CB)
    results = run(in_maps)
    out = np.empty((B, N, C), np.float32)
    for g in range(B):
        res = np.concatenate(
            [results[2 * g]["outd"], results[2 * g + 1]["outd"]], axis=0
        )
        out[g] = res[perms[g]]
    return out



# revision 20
# speedup vs baseline: 1.3456x; 1.3456x over previous
"""GCNConv Trainium2 kernel.

out = relu(deg^-1/2 ⊙ (A_w @ (deg^-1/2 ⊙ (x @ W)))) with self-loops, per graph.

Sharding: 8 cores = 4 graphs x 2 destination-row halves. Inside each core the
graph op is an SpMM executed as: dma_gather of y rows per edge (edges land on
partitions), a DVE-built w-valued one-hot scatter matrix per 128-edge chunk,
and a PE matmul accumulating each 128-row destination block in PSUM.
"""

import sys

sys.path.insert(0, "/opt/trn_rl_repo")

import numpy as np
import ml_dtypes

B, N, C, E = 4, 10000, 128, 160000
P = 128
NBLK = 80            # padded row blocks (10240 rows) for degree tables
NPAD = NBLK * P      # 10240
NBLK_Y = 79          # y table blocks (10112 rows >= N)
NPAD_Y = NBLK_Y * P  # 10112
HALF = 40            # destination blocks per core
HROWS = HALF * P     # 5120
GBLK = 1             # blocks per dma_gather batch

_COMPILED = {}       # (L, CB) -> (nc, in_names)


def _build(L, CB, repeat=1, rep_main_only=False, mode="full", nqueues=4,
           gblk=GBLK, single_packet=False, dt16=True, ydt32=False,
           gbufs=None, ohbufs=4, psbufs=2, xbufs=2):
    import concourse.bacc as bacc
    import concourse.mybir as mybir
    from concourse import tile

    dt = mybir.dt
    fdt = dt.bfloat16 if dt16 else dt.float32
    ydt = dt.float32 if ydt32 else fdt
    TC = HALF * CB * P          # padded edge slots per core
    NCHUNK = HALF * CB
    G_CH = gblk * CB            # chunks per gather batch
    NBATCH = HALF // gblk
    do_gather = mode in ("full", "gather_only")
    do_compute = mode in ("full", "compute_only")

    nc = bacc.Bacc(
        "TRN2", target_bir_lowering=True, debug=False, num_swdge_queues=nqueues
    )
    xT = nc.dram_tensor("xT", [P, NPAD_Y], fdt, kind="ExternalInput")
    wmat = nc.dram_tensor("wmat", [P, P], fdt, kind="ExternalInput")
    dinvc = nc.dram_tensor("dinvc", [P, NBLK], dt.float32, kind="ExternalInput")
    dinvl = nc.dram_tensor("dinvl", [P, HALF], dt.float32, kind="ExternalInput")
    idx16 = nc.dram_tensor("idx16", [P, TC // 16], dt.int16, kind="ExternalInput")
    rowloc = nc.dram_tensor("rowloc", [P, NCHUNK], dt.float32, kind="ExternalInput")
    wve = nc.dram_tensor("wve", [P, NCHUNK], dt.float32, kind="ExternalInput")
    iota = nc.dram_tensor("iota", [P, P], fdt, kind="ExternalInput")
    outd = nc.dram_tensor("outd", [HROWS, P], dt.float32, kind="ExternalOutput")

    with tile.TileContext(nc) as tc:
        with (
            tc.tile_pool(name="const", bufs=1) as cp,
            tc.tile_pool(name="ystage", bufs=2) as yp,
            tc.tile_pool(name="xstage", bufs=xbufs) as xp,
            tc.tile_pool(name="ostage", bufs=2) as op,
            tc.tile_pool(name="gather", bufs=(gbufs or max(2, 10 // gblk))) as gp,
            tc.tile_pool(name="onehot", bufs=ohbufs) as ohp,
            tc.tile_pool(name="psxw", bufs=4, space="PSUM") as pxw,
            tc.tile_pool(name="psmain", bufs=psbufs, space="PSUM") as pmain,
            tc.tile_pool(name="dram", bufs=1, space="DRAM") as dp,
        ):
            wmat_sb = cp.tile([P, P], fdt)
            idx_sb = cp.tile([P, TC // 16], dt.int16)
            rl_sb = cp.tile([P, NCHUNK], dt.float32)
            wve_sb = cp.tile([P, NCHUNK], dt.float32)
            iota_sb = cp.tile([P, P], fdt)
            dinv_c = cp.tile([P, NBLK], dt.float32)
            dinv_l = cp.tile([P, HALF], dt.float32)

            for rep in range(1 if rep_main_only else repeat):
                nc.sync.dma_start(out=wmat_sb[:], in_=wmat[:])
                nc.sync.dma_start(out=idx_sb[:], in_=idx16[:])
                nc.sync.dma_start(out=rl_sb[:], in_=rowloc[:])
                nc.sync.dma_start(out=wve_sb[:], in_=wve[:])
                nc.sync.dma_start(out=iota_sb[:], in_=iota[:])
                nc.sync.dma_start(out=dinv_c[:], in_=dinvc[:])
                nc.sync.dma_start(out=dinv_l[:], in_=dinvl[:])

                y_dram = dp.tile([NPAD_Y, P], ydt)

                # y = deg^-1/2 * (x @ W), written to DRAM in groups of 8 blocks
                for g0 in range(0, NBLK_Y, 8):
                    nb = min(8, NBLK_Y - g0)
                    ystage = yp.tile([P, 8 * P], ydt, tag="ystage")
                    xs = xp.tile([P, 8 * P], fdt, tag="xstage")
                    nc.sync.dma_start(
                        out=xs[:, : nb * P], in_=xT[:, g0 * P : (g0 + nb) * P]
                    )
                    for j in range(nb):
                        a = g0 + j
                        ps = pxw.tile([P, P], dt.float32)
                        nc.tensor.matmul(
                            ps[:],
                            lhsT=xs[:, j * P : (j + 1) * P],
                            rhs=wmat_sb[:],
                            start=True,
                            stop=True,
                        )
                        nc.scalar.activation(
                            ystage[:, j * P : (j + 1) * P],
                            ps[:],
                            mybir.ActivationFunctionType.Copy,
                            scale=dinv_c[:, a : a + 1],
                        )
                    nc.sync.dma_start(
                        out=y_dram[g0 * P : (g0 + nb) * P, :].rearrange(
                            "(j p) c -> p j c", p=P
                        ),
                        in_=ystage[:, : nb * P].rearrange("p (j c) -> p j c", c=P),
                    )

                # main loop: gather 2 blocks worth of edges at a time, scatter via
                # one-hot matmul into a per-block PSUM accumulator
                ostage = None
                gbuf = None
                if not do_gather:
                    gbuf = cp.tile([P, G_CH, P], ydt)
                    for j in range(G_CH):
                        nc.sync.dma_start(
                            out=gbuf[:, j, :],
                            in_=y_dram[j * P : (j + 1) * P, :],
                        )
                for _mrep in range(repeat if rep_main_only else 1):
                  for b in range(HALF):
                      if b % gblk == 0 and do_gather:
                          gbuf = gp.tile([P, G_CH, P], ydt, tag="gbuf")
                          bb = b // gblk
                          nc.gpsimd.dma_gather(
                              gbuf[:],
                              y_dram[:],
                              idx_sb[:, bb * G_CH * 8 : (bb + 1) * G_CH * 8],
                              G_CH * P,
                              G_CH * P,
                              P,
                              single_packet=single_packet,
                              queue_num=bb % nqueues,
                          )
                      if not do_compute:
                          continue
                      ps = pmain.tile([P, P], dt.float32, tag="psmain")
                      for k in range(CB):
                          ch = b * CB + k
                          slot = (b % gblk) * CB + k
                          oh = ohp.tile([P, P], fdt, tag="onehot")
                          nc.vector.tensor_scalar(
                              oh[:],
                              iota_sb[:],
                              rl_sb[:, ch : ch + 1],
                              wve_sb[:, ch : ch + 1],
                              mybir.AluOpType.is_equal,
                              mybir.AluOpType.mult,
                          )
                          nc.tensor.matmul(
                              ps[:],
                              lhsT=oh[:],
                              rhs=gbuf[:, slot, :],
                              start=(k == 0),
                              stop=(k == CB - 1),
                          )
                      if b % 8 == 0:
                          ostage = op.tile([P, 8 * P], dt.float32, tag="ostage")
                      nc.scalar.activation(
                          ostage[:, (b % 8) * P : (b % 8 + 1) * P],
                          ps[:],
                          mybir.ActivationFunctionType.Relu,
                          scale=dinv_l[:, b : b + 1],
                      )
                      if b % 8 == 7:
                          g0 = b - 7
                          nc.sync.dma_start(
                              out=outd[g0 * P : (g0 + 8) * P, :].rearrange(
                                  "(j p) c -> p j c", p=P
                              ),
                              in_=ostage[:].rearrange("p (j c) -> p j c", c=P),
                          )
    nc.compile()
    return nc


def _get(L, CB, repeat=1, rep_main_only=False, **kw):
    key = (L, CB, repeat, rep_main_only, tuple(sorted(kw.items())))
    if key not in _COMPILED:
        _COMPILED[key] = _build(L, CB, repeat, rep_main_only, **kw)
    return _COMPILED[key]


def _prep_inputs(x, edge_index, edge_weight, weight, dt16=True):
    """Host prep: per-graph balanced destination-row permutation + tables.

    Rows are assigned to the 80 destination blocks by in-degree serpentine
    (sorted desc, snake across blocks) so per-block edge counts are nearly
    equal -> CB (chunks per block) drops to ~ceil(mean/128). Sources within
    each block are sorted ascending for HBM gather locality.
    """
    bf16 = ml_dtypes.bfloat16
    fdt = bf16 if dt16 else np.float32
    x = np.asarray(x, np.float32)
    ei = np.asarray(edge_index)
    ew = np.asarray(edge_weight, np.float32)
    wt = np.asarray(weight, np.float32)

    loops = np.arange(N, dtype=np.int64)
    ones = np.ones(N, np.float32)
    NB = 2 * HALF

    graphs = []
    L_glob = 1
    CB_glob = 1
    for g in range(B):
        rows = np.concatenate([ei[g, 0], loops]).astype(np.int64)
        cols = np.concatenate([ei[g, 1], loops]).astype(np.int64)
        w = np.concatenate([ew[g], ones])

        counts = np.bincount(rows, minlength=N).astype(np.int64)
        L = int(counts.max())
        L_glob = max(L_glob, L)

        # serpentine balanced assignment of rows -> (block, slot)
        order = np.argsort(-counts, kind="stable")
        i = np.arange(N)
        rnd, pos = i // NB, i % NB
        blk_srt = np.where(rnd % 2 == 0, pos, NB - 1 - pos)
        blk_of_row = np.empty(N, np.int64)
        slot_of_row = np.empty(N, np.int64)
        blk_of_row[order] = blk_srt
        slot_of_row[order] = rnd

        eb = blk_of_row[rows]
        rl_full = slot_of_row[rows]

        halves = []
        for h in range(2):
            m = (eb >= h * HALF) & (eb < (h + 1) * HALF)
            lb = eb[m] - h * HALF
            hc = cols[m]
            hw = w[m]
            hrl = rl_full[m]
            o2 = np.lexsort((hc, lb))
            lb, hc, hw, hrl = lb[o2], hc[o2], hw[o2], hrl[o2]
            cnt = np.bincount(lb, minlength=HALF)
            CB_glob = max(CB_glob, int(np.ceil(cnt.max() / P)))
            halves.append((lb, hc, hw, hrl, cnt))
        graphs.append((rows, w, counts, halves, blk_of_row, slot_of_row))

    L, CB = L_glob, CB_glob
    TC = HALF * CB * P

    iota_np = np.tile(np.arange(P, dtype=np.float32), (P, 1)).astype(fdt)
    perms = []

    in_maps = []
    for g in range(B):
        rows, w, counts, halves, blk_of_row, slot_of_row = graphs[g]
        perms.append(blk_of_row * P + slot_of_row)
        # host-side weighted degree -> dinv tables
        deg = np.bincount(rows, weights=w, minlength=N).astype(np.float32)
        dinv = np.zeros(NPAD, np.float32)
        dinv[:N] = np.where(deg > 0, 1.0 / np.sqrt(deg), 0.0)
        dinv[N:] = 1.0
        dinvc = np.ascontiguousarray(dinv.reshape(NBLK, P).T)

        xp = np.zeros((NPAD_Y, P), np.float32)
        xp[:N] = x[g]
        xT = np.ascontiguousarray(xp.T).astype(fdt)

        for h in range(2):
            lb, hc, hw, hrl, cnt = halves[h]
            # permuted destination dinv table for dinv_l
            ris = np.full(HROWS, -1, np.int64)
            sel = (blk_of_row >= h * HALF) & (blk_of_row < (h + 1) * HALF)
            rr = np.nonzero(sel)[0]
            ris[(blk_of_row[rr] - h * HALF) * P + slot_of_row[rr]] = rr
            dl = np.ones(HROWS, np.float32)
            valid = ris >= 0
            dl[valid] = dinv[ris[valid]]
            dinvl = np.ascontiguousarray(dl.reshape(HALF, P).T)
            bstarts = np.zeros(HALF + 1, np.int64)
            np.cumsum(cnt, out=bstarts[1:])
            dst = lb * (CB * P) + (np.arange(lb.size, dtype=np.int64) - bstarts[lb])
            cols_pad = np.zeros(TC, np.int64)
            wv_pad = np.zeros(TC, np.float32)
            rl_pad = np.zeros(TC, np.float32)
            cols_pad[dst] = hc
            wv_pad[dst] = hw
            rl_pad[dst] = hrl.astype(np.float32)

            idx16 = np.tile(
                cols_pad.astype(np.int16).reshape(-1, 16).T, (8, 1)
            ).copy()
            rl_sb = np.ascontiguousarray(rl_pad.reshape(-1, P).T)
            wve_sb = np.ascontiguousarray(wv_pad.reshape(-1, P).T)

            in_maps.append(
                {
                    "xT": xT,
                    "wmat": wt.astype(fdt),
                    "dinvc": dinvc,
                    "dinvl": dinvl,
                    "idx16": idx16,
                    "rowloc": rl_sb,
                    "wve": wve_sb,
                    "iota": iota_np,
                }
            )
    return in_maps, L, CB, perms


_RUNNERS = {}


def _make_runner(nc):
    """Persistent jitted 8-core SPMD runner for a compiled Bass module.

    Mirrors bass2jax.run_bass_via_pjrt's multi-core path, but reusable
    across calls so repeated executions don't re-trace.
    """
    import jax
    import jax.numpy as jnp
    import concourse.mybir as mybir
    from jax.sharding import Mesh, PartitionSpec
    from jax.experimental.shard_map import shard_map
    from concourse.bass2jax import (
        _bass_exec_p,
        install_neuronx_cc_hook,
        partition_id_tensor,
    )

    install_neuronx_cc_hook()
    n_cores = 8
    pname = nc.partition_id_tensor.name if nc.partition_id_tensor else None
    in_names, out_names, out_avals = [], [], []
    for alloc in nc.m.functions[0].allocations:
        if not isinstance(alloc, mybir.MemoryLocationSet):
            continue
        name = alloc.memorylocations[0].name
        if alloc.kind == "ExternalInput":
            if name != pname:
                in_names.append(name)
        elif alloc.kind == "ExternalOutput":
            out_names.append(name)
            out_avals.append(
                jax.core.ShapedArray(
                    tuple(alloc.tensor_shape), mybir.dt.np(alloc.dtype)
                )
            )
    n_params = len(in_names)
    all_names = in_names + out_names
    if pname is not None:
        all_names = all_names + [pname]

    def _body(*args):
        operands = list(args)
        if pname is not None:
            operands.append(partition_id_tensor())
        return tuple(
            _bass_exec_p.bind(
                *operands,
                out_avals=tuple(out_avals),
                in_names=tuple(all_names),
                out_names=tuple(out_names),
                lowering_input_output_aliases=(),
                sim_require_finite=True,
                sim_require_nnan=True,
                nc=nc,
            )
        )

    devices = jax.devices()[:n_cores]
    mesh = Mesh(np.asarray(devices), ("core",))
    nz = len(out_avals)
    donate = tuple(range(n_params, n_params + nz))
    sharded = jax.jit(
        shard_map(
            _body,
            mesh=mesh,
            in_specs=(PartitionSpec("core"),) * (n_params + nz),
            out_specs=(PartitionSpec("core"),) * nz,
            check_rep=False,
        ),
        donate_argnums=donate,
        keep_unused=True,
    )

    def run(in_maps, want_np=True):
        concat_in = [
            np.concatenate([np.asarray(m[name]) for m in in_maps], axis=0)
            for name in in_names
        ]
        zeros = [
            jnp.zeros((n_cores * a.shape[0], *a.shape[1:]), a.dtype)
            for a in out_avals
        ]
        outs = sharded(*concat_in, *zeros)
        if not want_np:
            return outs
        return [
            {
                name: np.asarray(outs[i]).reshape(n_cores, *out_avals[i].shape)[c]
                for i, name in enumerate(out_names)
            }
            for c in range(n_cores)
        ]

    run.in_names = in_names
    run.out_avals = out_avals
    run.sharded = sharded
    run.n_params = n_params
    return run


def _get_runner(L, CB, repeat=1, **kw):
    key = (L, CB, repeat, tuple(sorted(kw.items())))
    if key not in _RUNNERS:
        _RUNNERS[key] = _make_runner(_get(L, CB, repeat, **kw))
    return _RUNNERS[key]


def kernel(x, edge_index, edge_weight, weight):
    in_maps, L, CB, perms = _prep_inputs(x, edge_index, edge_weight, weight)
    run = _get_runner(L, CB)
    results = run(in_maps)
    out = np.empty((B, N, C), np.float32)
    for g in range(B):
        res = np.concatenate(
            [results[2 * g]["outd"], results[2 * g + 1]["outd"]], axis=0
        )
        out[g] = res[perms[g]]
    return out

